# revision 34
# baseline (speedup 1.0000x reference)
"""Trainium2 Bass kernel for nn_MultiHeadLayer (pre-LN MHA, fused QKV).

Self-contained: takes FULL inputs, shards data-parallel over batch across
8 NeuronCores, runs a Bass/Tile kernel per core, gathers the full output.

Per-core dataflow (T = B_core*S tokens, H hidden, NH heads, D = H/NH):
  Phase 1: LN in natural layout -> PE-transpose -> xnT [H, T] (SBUF, f32r)
           stream qkv weight chunks, weight-stationary matmuls ->
           projT [3H, T] in a DRAM scratch pool (f32r)
  Phase 2: per (batch, head): scoresT = kT.T @ qT (k on partitions),
           exp fused with additive mask via per-partition ACT bias
           (no max subtraction: |scores| <~ 40 << 88 so exp is safe),
           sumexp broadcast via ones-matmul, ctxT = v.T-free matmul with
           normalization fused into the PSUM evacuation -> ctxT [H, T]
  Phase 3: outT = o.T @ ctxT, weight-stationary -> outT [H, T] -> host
           transposes during unshard.

All heavy matmuls use float32r (full PE rate at free dim >= 256,
~1e-4 relative error) with fp32 PSUM accumulation.
"""

import numpy as np
from functools import lru_cache

LN_EPS = 1e-5
NEG_BIG = -1.0e30


def _build(n_cores, T, S, H, NH, is_pre, has_bias, repeat=1):
    import concourse.bacc as bacc
    import concourse.mybir as mybir
    import concourse.tile as tile
    from concourse.masks import make_identity

    F32 = mybir.dt.float32
    F32R = mybir.dt.float32r
    BF16 = mybir.dt.bfloat16
    ACT = mybir.ActivationFunctionType

    KO = H // 128          # hidden-dim 128-chunks
    H3 = 3 * H
    D = H // NH
    DT = D // 128          # d-chunks per head
    KT = S // 128          # key-token 128-chunks per sequence
    B_core = T // S
    TC = T // 512          # token 512-chunks
    NCH = H3 // 128        # qkv column chunks of 128

    nc = bacc.Bacc("TRN2", target_bir_lowering=False, debug=False,
                   num_devices=n_cores)

    x_d = nc.dram_tensor("x", [T, H], F32, kind="ExternalInput")
    qkv_d = nc.dram_tensor("qkvw", [KO, 128, H3], BF16, kind="ExternalInput")
    o_d = nc.dram_tensor("ow", [KO, 128, H], F32R, kind="ExternalInput")
    # maskb[b*KT+kt, :] = additive key-mask bias for key tokens kt*128..+128
    mb_d = nc.dram_tensor("maskb", [B_core * KT, 128], F32,
                          kind="ExternalInput")
    if has_bias:
        # bqkv[i, :] = (bias @ qkvw)[i*128:(i+1)*128]
        bq_d = nc.dram_tensor("bqkv", [NCH, 128], F32, kind="ExternalInput")
    if is_pre:
        out_d = nc.dram_tensor("outT", [H, T], F32, kind="ExternalOutput")
    else:
        # post-LN needs LN params applied on-device to the output rows
        lnw_d = nc.dram_tensor("lnw", [H], F32, kind="ExternalInput")
        lnb_d = nc.dram_tensor("lnb", [H], F32, kind="ExternalInput")
        out_d = nc.dram_tensor("outN", [T, H], F32, kind="ExternalOutput")

    with tile.TileContext(nc) as tc:
        with tc.tile_pool(name="consts", bufs=1) as cp, \
             tc.tile_pool(name="dram", bufs=1, space="DRAM") as dp:
            ident = cp.tile([128, 128], F32)
            make_identity(nc, ident[:])
            identb = cp.tile([128, 128], BF16)
            nc.vector.tensor_copy(identb[:], ident[:])
            onesb = cp.tile([128, 128], BF16)
            nc.vector.memset(onesb[:], 1.0)
            eps_t = cp.tile([128, 1], F32)
            nc.vector.memset(eps_t[:], LN_EPS)
            mb_t = cp.tile([128, B_core * KT], F32)
            nc.sync.dma_start(mb_t[:], mb_d.ap().rearrange("i p -> p i"))
            if has_bias:
                bq_t = cp.tile([128, NCH], F32)
                nc.sync.dma_start(bq_t[:], bq_d.ap().rearrange("i p -> p i"))

            qkv_ap = qkv_d.ap().rearrange("ko p n -> p ko n")
            o_ap = o_d.ap().rearrange("ko p n -> p ko n")
            projT = dp.tile([H3, T], BF16)
            if not is_pre:
                oTs = dp.tile([H, T], F32)
                import concourse.bass as _bass
                lnw_bc = _bass.AP(tensor=lnw_d.ap().tensor, offset=0,
                                  ap=[[0, 128], [1, H]])
                lnb_bc = _bass.AP(tensor=lnb_d.ap().tensor, offset=0,
                                  ap=[[0, 128], [1, H]])
                lnw_t = cp.tile([128, H], F32)
                nc.sync.dma_start(lnw_t[:], lnw_bc)
                lnb_t = cp.tile([128, H], F32)
                nc.sync.dma_start(lnb_t[:], lnb_bc)

            for _rep in range(repeat):
                # ---------------- Phase 1: LN + transpose + QKV ----------------
                with tc.tile_pool(name="xnt", bufs=1) as xp, \
                     tc.tile_pool(name="ln", bufs=4) as lp, \
                     tc.tile_pool(name="lnsq", bufs=2) as sqp, \
                     tc.tile_pool(name="stats", bufs=24) as st, \
                     tc.tile_pool(name="tps", bufs=2, space="PSUM") as tps, \
                     tc.tile_pool(name="wch", bufs=3) as wp, \
                     tc.tile_pool(name="ev1", bufs=3) as ep, \
                     tc.tile_pool(name="ps1", bufs=2, space="PSUM") as pp1:
                    # Two half-token tiles so QKV matmuls on the first 512
                    # tokens only depend on the first 4 LN+transpose tiles.
                    # bf16 throughout QKV: same PE stream rate as f32r, half
                    # the SBUF/DMA, and fast (FWL) weight loads.
                    xnTs = [xp.tile([128, KO, 512], BF16, name=f"xnT{i}")
                            for i in range(TC)]
                    lns = {}

                    def ln_a(tt):
                        # stage A: DMA + row-sum + centered sum-of-squares
                        xt = lp.tile([128, H], F32)
                        nc.sync.dma_start(
                            xt[:], x_d.ap()[tt * 128:(tt + 1) * 128, :])
                        d = dict(xt=xt)
                        if is_pre:
                            ssum = st.tile([128, 1], F32)
                            nc.vector.reduce_sum(out=ssum[:], in_=xt[:],
                                                 axis=mybir.AxisListType.X)
                            negmu = st.tile([128, 1], F32)
                            nc.vector.tensor_scalar_mul(negmu[:], ssum[:],
                                                        -1.0 / H)
                            vs = []
                            for c in range(2):
                                xsq = sqp.tile([128, H // 2], F32, tag="xsq")
                                v = st.tile([128, 1], F32)
                                nc.scalar.activation(
                                    xsq[:], xt[:, c * (H // 2):(c + 1) * (H // 2)],
                                    ACT.Square, bias=negmu[:], scale=1.0,
                                    accum_out=v[:])
                                vs.append(v)
                            vsum = st.tile([128, 1], F32)
                            nc.scalar.add(vsum[:], vs[0][:], add=vs[1][:])
                            sd = st.tile([128, 1], F32)
                            nc.scalar.activation(sd[:], vsum[:], ACT.Sqrt,
                                                 bias=eps_t[:], scale=1.0 / H)
                            d.update(negmu=negmu, sd=sd)
                        lns[tt] = d

                    def ln_b(tt):
                        # stage B: rstd + normalize + transpose into xnTs
                        d = lns.pop(tt)
                        xt = d["xt"]
                        if is_pre:
                            rstd = st.tile([128, 1], F32)
                            nc.vector.reciprocal(rstd[:], d["sd"][:])
                            nc.vector.tensor_scalar(
                                out=xt[:], in0=xt[:],
                                scalar1=d["negmu"][:], scalar2=rstd[:],
                                op0=mybir.AluOpType.add,
                                op1=mybir.AluOpType.mult)
                        half, loc = tt // 4, (tt % 4) * 128
                        for hh in range(KO):
                            pt = tps.tile([128, 128], F32)
                            nc.tensor.transpose(
                                pt[:], xt[:, hh * 128:(hh + 1) * 128],
                                ident[:])
                            if hh % 2 == 0:
                                nc.vector.tensor_copy(
                                    xnTs[half][:, hh, loc:loc + 128], pt[:])
                            else:
                                nc.scalar.copy(
                                    xnTs[half][:, hh, loc:loc + 128], pt[:])

                    qsc = float(1.0 / np.sqrt(H // NH))

                    def qkv_chunk(nch, tchs):
                        wt = wp.tile([128, KO, 128], BF16)
                        nc.sync.dma_start(
                            wt[:], qkv_ap[:, :, nch * 128:(nch + 1) * 128])
                        for tch in tchs:
                            ps = pp1.tile([128, 512], F32, tag=f"ps1_{tch}",
                                          name=f"ps1_{tch}")
                            for ko in range(KO):
                                nc.tensor.matmul(
                                    ps[:], wt[:, ko], xnTs[tch][:, ko, :],
                                    start=(ko == 0), stop=(ko == KO - 1))
                            ev = ep.tile([128, 512], BF16)
                            if has_bias and nch * 128 < H:
                                nc.vector.tensor_scalar(
                                    out=ev[:], in0=ps[:], scalar1=qsc,
                                    scalar2=bq_t[:, nch:nch + 1],
                                    op0=mybir.AluOpType.mult,
                                    op1=mybir.AluOpType.add)
                            elif has_bias:
                                nc.vector.tensor_scalar_add(
                                    ev[:], ps[:], bq_t[:, nch:nch + 1])
                            elif nch * 128 < H:
                                nc.vector.tensor_scalar_mul(ev[:], ps[:], qsc)
                            else:
                                nc.vector.tensor_copy(ev[:], ps[:])
                            nc.sync.dma_start(
                                projT[nch * 128:(nch + 1) * 128,
                                      tch * 512:(tch + 1) * 512], ev[:])

                    # software-pipelined LN (A one tile ahead of B), with
                    # early QKV chunks interleaved to keep the PE busy while
                    # the second half's LN stats run on DVE/ACT.
                    NT = T // 128
                    ln_a(0)
                    ln_a(1)
                    ln_b(0)
                    ln_a(2)
                    ln_b(1)
                    ln_a(3)
                    ln_b(2)
                    ln_b(3)                      # xnTs[0] complete
                    ln_a(4)
                    qkv_chunk(0, [0])
                    ln_a(5)
                    ln_b(4)
                    qkv_chunk(1, [0])
                    ln_a(6)
                    ln_b(5)
                    qkv_chunk(2, [0])
                    ln_a(7)
                    ln_b(6)
                    qkv_chunk(3, [0])
                    ln_b(7)                      # xnTs[1] complete
                    for nch in range(4):
                        qkv_chunk(nch, [1])
                    for nch in range(4, NCH):
                        qkv_chunk(nch, [0, 1])

                # ---------------- Phase 2: attention ----------------
                with tc.tile_pool(name="ctxt", bufs=1) as cxp:
                    # Half-token tiles: phase 3 on tokens 0-511 (batches 0-1)
                    # starts while attention runs batches 2-3.
                    ctxTs = [cxp.tile([128, KO, 512], F32R, name=f"ctxT{i}")
                             for i in range(TC)]
                    assert NH % 2 == 0
                    with tc.tile_pool(name="ld2", bufs=6) as ld, \
                         tc.tile_pool(name="vna", bufs=4) as vp, \
                         tc.tile_pool(name="exp2", bufs=2) as xpp, \
                         tc.tile_pool(name="rec2", bufs=2) as rp, \
                         tc.tile_pool(name="ps2s", bufs=2, space="PSUM") as p2s, \
                         tc.tile_pool(name="ps2m", bufs=2, space="PSUM") as p2m, \
                         tc.tile_pool(name="ps2c", bufs=2, space="PSUM") as p2c, \
                         tc.tile_pool(name="tps2", bufs=2, space="PSUM") as tp2:
                        # Heads in pairs packed side-by-side into 512-wide
                        # PSUM banks. Software-pipelined across pairs so the
                        # PE never waits on the ACT exp / DVE recip chain:
                        # loads 2 ahead, transposes+scores+exp 1 ahead,
                        # sum+recip+ctx for the current pair.
                        pairs = [(b, p) for b in range(B_core)
                                 for p in range(NH // 2)]
                        stt = {}

                        def emit_load(i):
                            b, p = pairs[i]
                            heads = (2 * p, 2 * p + 1)
                            qT, kT, vT = [], [], []
                            for n in heads:
                                q_ = ld.tile([128, DT, S], BF16, tag="qT")
                                k_ = ld.tile([128, DT, S], BF16, tag="kT")
                                v_ = ld.tile([128, DT, S], BF16, tag="vT")
                                for dt in range(DT):
                                    r0 = n * D + dt * 128
                                    nc.sync.dma_start(
                                        q_[:, dt],
                                        projT[r0:r0 + 128, b * S:(b + 1) * S])
                                    nc.sync.dma_start(
                                        k_[:, dt],
                                        projT[H + r0:H + r0 + 128,
                                              b * S:(b + 1) * S])
                                    nc.sync.dma_start(
                                        v_[:, dt],
                                        projT[2 * H + r0:2 * H + r0 + 128,
                                              b * S:(b + 1) * S])
                                qT.append(q_)
                                kT.append(k_)
                                vT.append(v_)
                            stt[i] = dict(b=b, heads=heads, qT=qT, kT=kT,
                                          vT=vT)

                        def emit_produce(i):
                            st = stt[i]
                            b = st["b"]
                            vn = []
                            for h in range(2):
                                vn_ = vp.tile([128, KT, D], BF16, tag="vn")
                                for kt in range(KT):
                                    for dt in range(DT):
                                        pt = tp2.tile([128, 128], BF16)
                                        nc.tensor.transpose(
                                            pt[:],
                                            st["vT"][h][:, dt,
                                                        kt * 128:(kt + 1) * 128],
                                            identb[:])
                                        nc.vector.tensor_copy(
                                            vn_[:, kt, dt * 128:(dt + 1) * 128],
                                            pt[:])
                                vn.append(vn_)
                            expT = xpp.tile([128, KT, 2 * S], BF16, tag="expT")
                            for kt in range(KT):
                                pss = p2s.tile([128, 2 * S], F32)
                                for h in range(2):
                                    for dt in range(DT):
                                        nc.tensor.matmul(
                                            pss[:, h * S:(h + 1) * S],
                                            st["kT"][h][:, dt,
                                                        kt * 128:(kt + 1) * 128],
                                            st["qT"][h][:, dt],
                                            start=(dt == 0),
                                            stop=(dt == DT - 1))
                                nc.scalar.activation(
                                    expT[:, kt], pss[:], ACT.Exp,
                                    bias=mb_t[:, b * KT + kt:b * KT + kt + 1],
                                    scale=1.0)
                            st["vn"] = vn
                            st["expT"] = expT

                        def emit_consume(i):
                            st = stt.pop(i)
                            b, heads = st["b"], st["heads"]
                            expT, vn = st["expT"], st["vn"]
                            psm = p2m.tile([128, 2 * S], F32)
                            for kt in range(KT):
                                nc.tensor.matmul(psm[:], onesb[:], expT[:, kt],
                                                 start=(kt == 0),
                                                 stop=(kt == KT - 1))
                            rec = rp.tile([128, 2 * S], F32)
                            nc.vector.reciprocal(rec[:], psm[:])
                            for dt in range(DT):
                                psc = p2c.tile([128, 2 * S], F32)
                                for h in range(2):
                                    for kt in range(KT):
                                        nc.tensor.matmul(
                                            psc[:, h * S:(h + 1) * S],
                                            vn[h][:, kt, dt * 128:(dt + 1) * 128],
                                            expT[:, kt, h * S:(h + 1) * S],
                                            start=(kt == 0), stop=(kt == KT - 1))
                                for h in range(2):
                                    nc.vector.tensor_tensor(
                                        ctxTs[b // 2][:, heads[h] * DT + dt,
                                                      (b % 2) * S:
                                                      (b % 2 + 1) * S],
                                        psc[:, h * S:(h + 1) * S],
                                        rec[:, h * S:(h + 1) * S],
                                        mybir.AluOpType.mult)

                        NPAIR = len(pairs)
                        emit_load(0)
                        if NPAIR > 1:
                            emit_load(1)
                        emit_produce(0)
                        for i in range(NPAIR):
                            if i + 2 < NPAIR:
                                emit_load(i + 2)
                            if i + 1 < NPAIR:
                                emit_produce(i + 1)
                            emit_consume(i)

                    # ---------------- Phase 3: output projection ----------------
                    with tc.tile_pool(name="och", bufs=3) as op_, \
                         tc.tile_pool(name="ev3", bufs=3) as e3, \
                         tc.tile_pool(name="ps3", bufs=2, space="PSUM") as pp3:
                        for hoch in range(KO):
                            ot = op_.tile([128, KO, 128], F32R)
                            nc.sync.dma_start(
                                ot[:], o_ap[:, :, hoch * 128:(hoch + 1) * 128])
                            psl = [pp3.tile([128, 512], F32, tag=f"ps3_{t}",
                                            name=f"ps3_{t}")
                                   for t in range(TC)]
                            for tch in range(TC):
                                for ko in range(KO):
                                    nc.tensor.matmul(
                                        psl[tch][:], ot[:, ko],
                                        ctxTs[tch][:, ko, :],
                                        start=(ko == 0), stop=(ko == KO - 1))
                            for tch in range(TC):
                                ps = psl[tch]
                                ev = e3.tile([128, 512], F32)
                                nc.vector.tensor_copy(ev[:], ps[:])
                                dst = (out_d.ap() if is_pre else oTs)
                                nc.sync.dma_start(
                                    dst[hoch * 128:(hoch + 1) * 128,
                                        tch * 512:(tch + 1) * 512], ev[:])

                # ---------------- Phase 4 (isPre=0): transpose + post-LN -------
                if not is_pre:
                    with tc.tile_pool(name="p4in", bufs=3) as p4i, \
                         tc.tile_pool(name="p4out", bufs=2) as p4o, \
                         tc.tile_pool(name="st4", bufs=8) as st4, \
                         tc.tile_pool(name="sq4", bufs=2) as sq4, \
                         tc.tile_pool(name="tps4", bufs=4, space="PSUM") as tp4:
                        for tt in range(T // 128):
                            on = p4o.tile([128, H], F32)
                            for hh in range(KO):
                                it = p4i.tile([128, 128], F32)
                                nc.sync.dma_start(
                                    it[:], oTs[hh * 128:(hh + 1) * 128,
                                               tt * 128:(tt + 1) * 128])
                                pt = tp4.tile([128, 128], F32)
                                nc.tensor.transpose(pt[:], it[:], ident[:])
                                nc.vector.tensor_copy(
                                    on[:, hh * 128:(hh + 1) * 128], pt[:])
                            ssum = st4.tile([128, 1], F32)
                            nc.vector.reduce_sum(out=ssum[:], in_=on[:],
                                                 axis=mybir.AxisListType.X)
                            negmu = st4.tile([128, 1], F32)
                            nc.vector.tensor_scalar_mul(negmu[:], ssum[:], -1.0 / H)
                            xsq = sq4.tile([128, H], F32)
                            vsum = st4.tile([128, 1], F32)
                            nc.scalar.activation(xsq[:], on[:], ACT.Square,
                                                 bias=negmu[:], scale=1.0,
                                                 accum_out=vsum[:])
                            sd = st4.tile([128, 1], F32)
                            nc.scalar.activation(sd[:], vsum[:], ACT.Sqrt,
                                                 bias=eps_t[:], scale=1.0 / H)
                            rstd = st4.tile([128, 1], F32)
                            nc.vector.reciprocal(rstd[:], sd[:])
                            nc.vector.tensor_scalar(
                                out=on[:], in0=on[:],
                                scalar1=negmu[:], scalar2=rstd[:],
                                op0=mybir.AluOpType.add,
                                op1=mybir.AluOpType.mult)
                            nc.vector.tensor_tensor(on[:], on[:], lnw_t[:],
                                                    mybir.AluOpType.mult)
                            nc.vector.tensor_tensor(on[:], on[:], lnb_t[:],
                                                    mybir.AluOpType.add)
                            nc.sync.dma_start(
                                out_d.ap()[tt * 128:(tt + 1) * 128, :], on[:])

    nc.finalize()
    return nc


@lru_cache(maxsize=4)
def _get_runner(n_cores, T, S, H, NH, is_pre, has_bias, repeat=1):
    """Build + jit once; returns fn(in_maps) -> list of out dicts."""
    import jax
    import numpy as _np
    from jax.sharding import Mesh, PartitionSpec
    from jax.experimental.shard_map import shard_map
    import concourse.mybir as mybir
    from concourse import bass2jax
    from concourse.bass2jax import _bass_exec_p, install_neuronx_cc_hook

    nc = _build(n_cores, T, S, H, NH, is_pre, has_bias, repeat)
    install_neuronx_cc_hook()

    partition_name = (nc.partition_id_tensor.name
                      if nc.partition_id_tensor else None)
    in_names, out_names, out_avals, zero_shapes = [], [], [], []
    for alloc in nc.m.functions[0].allocations:
        if not isinstance(alloc, mybir.MemoryLocationSet):
            continue
        name = alloc.memorylocations[0].name
        if alloc.kind == "ExternalInput":
            if name != partition_name:
                in_names.append(name)
        elif alloc.kind == "ExternalOutput":
            out_names.append(name)
            shape = tuple(alloc.tensor_shape)
            dtype = mybir.dt.np(alloc.dtype)
            out_avals.append(jax.core.ShapedArray(shape, dtype))
            zero_shapes.append((shape, dtype))
    n_params = len(in_names)
    n_outs = len(out_avals)
    all_in_names = list(in_names) + list(out_names)
    if partition_name is not None:
        all_in_names.append(partition_name)

    def _body(*args):
        operands = list(args)
        if partition_name is not None:
            operands.append(bass2jax.partition_id_tensor())
        outs = _bass_exec_p.bind(
            *operands,
            out_avals=tuple(out_avals),
            in_names=tuple(all_in_names),
            out_names=tuple(out_names),
            lowering_input_output_aliases=(),
            sim_require_finite=True,
            sim_require_nnan=True,
            nc=nc,
        )
        return tuple(outs)

    devices = jax.devices()[:n_cores]
    if n_cores == 1:
        jfn = jax.jit(_body, keep_unused=True)

        def _prep(in_maps):
            args = [jax.device_put(_np.asarray(in_maps[0][n]))
                    for n in in_names]
            zeros = [jax.device_put(_np.zeros(s, d)) for s, d in zero_shapes]
            return args + zeros

        def _collect(outs):
            return [{n: _np.asarray(outs[i]) for i, n in enumerate(out_names)}]
    else:
        mesh = Mesh(np.asarray(devices), ("core",))
        from jax.sharding import NamedSharding
        shard = NamedSharding(mesh, PartitionSpec("core"))
        repl = NamedSharding(mesh, PartitionSpec())
        REPLICATED = {"qkvw", "ow", "bqkv", "lnw", "lnb"}
        in_specs = tuple(
            (PartitionSpec() if n in REPLICATED else PartitionSpec("core"))
            for n in in_names) + (PartitionSpec("core"),) * n_outs
        out_specs = (PartitionSpec("core"),) * n_outs
        jfn = jax.jit(
            shard_map(_body, mesh=mesh, in_specs=in_specs,
                      out_specs=out_specs, check_rep=False),
            keep_unused=True)

        def _prep(in_maps):
            concat_in = []
            for n in in_names:
                if n in REPLICATED:
                    concat_in.append(
                        jax.device_put(_np.asarray(in_maps[0][n]), repl))
                else:
                    concat_in.append(jax.device_put(
                        _np.concatenate([_np.asarray(m[n]) for m in in_maps],
                                        axis=0), shard))
            zeros = [
                jax.device_put(
                    _np.zeros((n_cores * s[0], *s[1:]), d), shard)
                for s, d in zero_shapes]
            return concat_in + zeros

        def _collect(outs):
            return [
                {n: _np.asarray(outs[i]).reshape(
                    n_cores, *out_avals[i].shape)[c]
                 for i, n in enumerate(out_names)}
                for c in range(n_cores)]

    class Runner:
        in_names_ = in_names
        out_names_ = out_names

        def prep(self, in_maps):
            return _prep(in_maps)

        def call(self, args):
            return jfn(*args)

        def run(self, in_maps):
            outs = jfn(*_prep(in_maps))
            jax.block_until_ready(outs)
            return _collect(outs)

        def collect(self, outs):
            return _collect(outs)

    return Runner()


def _prep_core_inputs(inp, mask, weight, bias, qkv, o, is_pre, n_cores,
                      NH=16):
    """Host-side prep: fold LN weight + 1/sqrt(D) into qkv, build per-core
    input dicts."""
    B, S, H = inp.shape
    D = H // NH
    B_core = B // n_cores
    T = B_core * S
    KO = H // 128
    H3 = 3 * H
    KT = S // 128

    # Pre-LN: xn = z*w + b with z the normalized input, so
    # xn @ qkv = (z) @ (w[:,None]*qkv) + (b @ qkv): fold w into the weights
    # and b into a per-output-channel additive term applied on-device.
    # The 1/sqrt(D) query scale is applied on-device in the PSUM
    # evacuation, so with w==1 and b==0 the weights pass through zero-copy.
    qkvw = qkv.astype(np.float32)
    if is_pre:
        w = weight.astype(np.float32)
        if not np.all(w == 1.0):
            qkvw = qkvw * w[:, None]
        bqkv = bias.astype(np.float32) @ qkv.astype(np.float32)
    else:
        bqkv = np.zeros(H3, dtype=np.float32)
    bqkv[:H] *= np.float32(1.0 / np.sqrt(D))
    has_bias = bool(np.any(bqkv))

    import ml_dtypes
    qkv_r = qkvw.astype(ml_dtypes.bfloat16).reshape(KO, 128, H3)
    o_r = o.astype(np.float32).reshape(KO, 128, H)

    maskbias = np.where(mask != 0, np.float32(NEG_BIG), np.float32(0.0))
    maskbias = maskbias.astype(np.float32)  # [B, S]

    in_maps = []
    for c in range(n_cores):
        xb = inp[c * B_core:(c + 1) * B_core].reshape(T, H)
        mb = maskbias[c * B_core:(c + 1) * B_core].reshape(B_core * KT, 128)
        m = {
            "x": np.ascontiguousarray(xb.astype(np.float32)),
            "qkvw": qkv_r,
            "ow": o_r,
            "maskb": np.ascontiguousarray(mb),
        }
        if has_bias:
            m["bqkv"] = np.ascontiguousarray(
                bqkv.reshape(H3 // 128, 128))
        if not is_pre:
            m["lnw"] = np.ascontiguousarray(weight.astype(np.float32))
            m["lnb"] = np.ascontiguousarray(bias.astype(np.float32))
        in_maps.append(m)
    return in_maps, has_bias, (B, S, H, NH, B_core, T)


def kernel(inp, mask, weight, bias, qkv, o, isPre):
    inp = np.asarray(inp)
    mask = np.asarray(mask)
    weight = np.asarray(weight)
    bias = np.asarray(bias)
    qkv = np.asarray(qkv)
    o = np.asarray(o)
    is_pre = bool(int(np.asarray(isPre)))

    n_cores = 8
    NH = 16
    in_maps, has_bias, (B, S, H, _, B_core, T) = _prep_core_inputs(
        inp, mask, weight, bias, qkv, o, is_pre, n_cores)

    runner = _get_runner(n_cores, T, S, H, NH, is_pre, has_bias)
    results = runner.run(in_maps)

    out = np.empty((B, S, H), dtype=np.float32)
    for c in range(n_cores):
        if is_pre:
            outT = results[c]["outT"]  # [H, T]
            out[c * B_core:(c + 1) * B_core] = outT.T.reshape(B_core, S, H)
        else:
            out[c * B_core:(c + 1) * B_core] = (
                results[c]["outN"].reshape(B_core, S, H))
    return out

